# revision 7
# baseline (speedup 1.0000x reference)
"""GraphUpsampling kernel for 8x TRN2 NeuronCores.

Math: out = (A / colsum(A)) @ input.reshape(P,C)[descendance]
    == A @ (up / colsum(A)[:,None])          (scale the small side)

Sharding: COLUMN-shard A across 8 cores. Core k owns columns
j in [k*1024, (k+1)*1024). Each core holds the FULL column, so it
computes its own colsum locally -- zero communication. Each core
produces a partial output (8192, 32) = A[:, jk] @ up_scaled[jk]; the
host sums the 8 partials (the unshard reduction).

Device layout: core k's A slice is pre-transposed on host to
at = A[:, jk].T with shape (1024, 8192), so the contraction dim j is
the SBUF partition dim -- no on-chip transpose needed, colsum is a
free-dim vector reduce, and matmul uses at tiles as stationary lhsT.

PSUM: the full partial output (64 i-blocks x [128, 32]) is packed into
4 PSUM banks. A start=True matmul clears has_written bits bank-wide,
so we zero each bank once with a cheap K=1 all-zeros matmul and run
every real matmul with start=False (pure accumulate).
"""

import sys

sys.path.insert(0, "/opt/trn_rl_repo")

import numpy as np

import concourse.bass as bass
import concourse.mybir as mybir
from concourse import bacc
from concourse.bass_utils import run_bass_kernel_spmd
from concourse.tile import TileContext

PARENT = 4096
CHILD = 8192
C = 32
NCORES = 8
JPC = CHILD // NCORES  # 1024 columns of A per core
NSTRIPE = JPC // 128  # 8 stripes of 128 j per core
NIB = CHILD // 128  # 64 i-blocks of 128

_CACHE = {}


def _build_program():
    fp32 = mybir.dt.float32
    nc = bacc.Bacc("TRN2", target_bir_lowering=False)
    at = nc.dram_tensor("at", (JPC, CHILD), fp32, kind="ExternalInput")
    u = nc.dram_tensor("u", (JPC, C), fp32, kind="ExternalInput")
    # Output in scrambled layout [128, 64*32]: y2[p, ib*32+c] = Y[ib*128+p, c].
    # Host unscrambles; this keeps the store DMA contiguous (8KB/partition).
    y2 = nc.dram_tensor("y2", (128, NIB * C), fp32, kind="ExternalOutput")

    with TileContext(nc) as tc:
        with (
            tc.tile_pool(name="stripes", bufs=3) as spool,
            tc.tile_pool(name="small", bufs=1) as small,
            tc.tile_pool(name="uscaled", bufs=NSTRIPE) as upool,
            tc.tile_pool(name="stats", bufs=NSTRIPE) as stpool,
            tc.tile_pool(name="psum", bufs=1, space="PSUM") as ppool,
            tc.tile_pool(name="evict", bufs=1) as epool,
        ):
            zlhs = small.tile([1, 128], fp32, tag="zlhs")
            nc.vector.memset(zlhs, 0.0)
            zrhs = small.tile([1, 512], fp32, tag="zrhs")
            nc.vector.memset(zrhs, 0.0)

            psum_out = ppool.tile([128, NIB * C], fp32)  # 2048 fp32 = 4 banks
            # Zero all 4 banks + set every has_written bit (K=1 matmul of zeros).
            for b in range(4):
                nc.tensor.matmul(
                    psum_out[:, b * 512 : (b + 1) * 512],
                    zlhs[:, :],
                    zrhs[:, :],
                    start=True,
                    stop=False,
                    skip_group_check=True,
                )

            for jc in range(NSTRIPE):
                stripe = spool.tile([128, CHILD], fp32, tag="stripe")
                nc.sync.dma_start(stripe, at[jc * 128 : (jc + 1) * 128, :])
                s = stpool.tile([128, 1], fp32, tag="s")
                nc.vector.reduce_sum(s, stripe, axis=mybir.AxisListType.X)
                r = stpool.tile([128, 1], fp32, tag="r")
                nc.vector.reciprocal(r, s)
                uc = upool.tile([128, C], fp32, tag="uc")
                nc.sync.dma_start(uc, u[jc * 128 : (jc + 1) * 128, :])
                us = upool.tile([128, C], fp32, tag="us")
                nc.scalar.activation(
                    us, uc, mybir.ActivationFunctionType.Copy, scale=r
                )
                last = jc == NSTRIPE - 1
                for ib in range(NIB):
                    nc.tensor.matmul(
                        psum_out[:, ib * C : (ib + 1) * C],
                        stripe[:, ib * 128 : (ib + 1) * 128],
                        us[:, :],
                        start=False,
                        stop=last,
                        skip_group_check=True,
                    )

            out_sb = epool.tile([128, NIB * C], fp32)
            nc.scalar.copy(out_sb, psum_out)
            nc.sync.dma_start(y2[:, :], out_sb)

    nc.finalize()
    return nc


def kernel(input, A, descendance):
    input = np.asarray(input)
    A = np.asarray(A, dtype=np.float32)
    desc = np.asarray(descendance).astype(np.int64)

    matrix_in = np.ascontiguousarray(input, dtype=np.float32).reshape(PARENT, C)
    up = matrix_in[desc]  # (CHILD, C) gather

    if "nc" not in _CACHE:
        _CACHE["nc"] = _build_program()
    nc = _CACHE["nc"]

    # Shard: core k gets at = A[:, k*JPC:(k+1)*JPC].T  (contiguous (JPC, CHILD))
    at_all = np.ascontiguousarray(
        A.reshape(CHILD, NCORES, JPC).transpose(1, 2, 0)
    )  # (NCORES, JPC, CHILD)
    in_maps = []
    for k in range(NCORES):
        in_maps.append(
            {
                "at": at_all[k],
                "u": np.ascontiguousarray(up[k * JPC : (k + 1) * JPC]),
            }
        )

    res = run_bass_kernel_spmd(nc, in_maps, core_ids=list(range(NCORES)))
    outs = res.results

    acc = np.zeros((128, NIB * C), dtype=np.float64)
    for k in range(NCORES):
        acc += outs[k]["y2"]
    # Unscramble: y2[p, ib*32+c] -> Y[ib*128+p, c]
    Y = (
        acc.reshape(128, NIB, C)
        .transpose(1, 0, 2)
        .reshape(CHILD, C)
        .astype(np.float32)
    )
    return Y.reshape(1, C, CHILD)


# revision 9
# speedup vs baseline: 18.2121x; 18.2121x over previous
"""GraphUpsampling kernel for 8x TRN2 NeuronCores.

Math: out = (A / colsum(A)) @ input.reshape(P,C)[descendance]
    == A @ (up / colsum(A)[:,None])          (scale the small side)

Sharding: COLUMN-shard A across 8 cores. Core k owns columns
j in [k*1024, (k+1)*1024). Each core holds the FULL column, so it
computes its own colsum locally -- zero communication. Each core
produces a partial output (8192, 32) = A[:, jk] @ up_scaled[jk]; the
host sums the 8 partials (the unshard reduction).

Device layout: core k's A slice is pre-transposed on host to
at = A[:, jk].T with shape (1024, 8192), so the contraction dim j is
the SBUF partition dim -- no on-chip transpose needed, colsum is a
free-dim vector reduce, and matmul uses at tiles as stationary lhsT.

PSUM: the full partial output (64 i-blocks x [128, 32]) is packed into
4 PSUM banks. A start=True matmul clears has_written bits bank-wide,
so we zero each bank once with a cheap K=1 all-zeros matmul and run
every real matmul with start=False (pure accumulate).
"""

import sys

sys.path.insert(0, "/opt/trn_rl_repo")

import numpy as np

import concourse.bass as bass
import concourse.mybir as mybir
from concourse import bacc
from concourse.bass_utils import run_bass_kernel_spmd
from concourse.tile import TileContext

PARENT = 4096
CHILD = 8192
C = 32
NCORES = 8
JPC = CHILD // NCORES  # 1024 columns of A per core
NSTRIPE = JPC // 128  # 8 stripes of 128 j per core
NIB = CHILD // 128  # 64 i-blocks of 128

_CACHE = {}


def _build_program(repeats=1):
    fp32 = mybir.dt.float32
    nc = bacc.Bacc("TRN2", target_bir_lowering=False)
    at = nc.dram_tensor("at", (JPC, CHILD), fp32, kind="ExternalInput")
    u = nc.dram_tensor("u", (JPC, C), fp32, kind="ExternalInput")
    # Output in scrambled layout [128, 64*32]: y2[p, ib*32+c] = Y[ib*128+p, c].
    # Host unscrambles; this keeps the store DMA contiguous (8KB/partition).
    y2 = nc.dram_tensor("y2", (128, NIB * C), fp32, kind="ExternalOutput")

    with TileContext(nc) as tc:
        with (
            tc.tile_pool(name="stripes", bufs=3) as spool,
            tc.tile_pool(name="small", bufs=1) as small,
            tc.tile_pool(name="uscaled", bufs=NSTRIPE) as upool,
            tc.tile_pool(name="stats", bufs=NSTRIPE) as stpool,
            tc.tile_pool(name="psum", bufs=1, space="PSUM") as ppool,
            tc.tile_pool(name="evict", bufs=1) as epool,
        ):
            zlhs = small.tile([1, 128], fp32, tag="zlhs")
            nc.vector.memset(zlhs, 0.0)
            zrhs = small.tile([1, 512], fp32, tag="zrhs")
            nc.vector.memset(zrhs, 0.0)

            for rep in range(repeats):
                psum_out = ppool.tile([128, NIB * C], fp32)  # 2048 fp32 = 4 banks
                # Zero all 4 banks + set every has_written bit (K=1 matmul).
                for b in range(4):
                    nc.tensor.matmul(
                        psum_out[:, b * 512 : (b + 1) * 512],
                        zlhs[:, :],
                        zrhs[:, :],
                        start=True,
                        stop=False,
                        skip_group_check=True,
                    )

                for jc in range(NSTRIPE):
                    stripe = spool.tile([128, CHILD], fp32, tag="stripe")
                    nc.sync.dma_start(stripe, at[jc * 128 : (jc + 1) * 128, :])
                    s = stpool.tile([128, 1], fp32, tag="s")
                    nc.vector.reduce_sum(s, stripe, axis=mybir.AxisListType.X)
                    r = stpool.tile([128, 1], fp32, tag="r")
                    nc.vector.reciprocal(r, s)
                    uc = upool.tile([128, C], fp32, tag="uc")
                    nc.sync.dma_start(uc, u[jc * 128 : (jc + 1) * 128, :])
                    us = upool.tile([128, C], fp32, tag="us")
                    nc.scalar.activation(
                        us, uc, mybir.ActivationFunctionType.Copy, scale=r
                    )
                    last = jc == NSTRIPE - 1
                    for ib in range(NIB):
                        nc.tensor.matmul(
                            psum_out[:, ib * C : (ib + 1) * C],
                            stripe[:, ib * 128 : (ib + 1) * 128],
                            us[:, :],
                            start=False,
                            stop=last,
                            skip_group_check=True,
                        )

                out_sb = epool.tile([128, NIB * C], fp32)
                nc.scalar.copy(out_sb, psum_out)
                nc.sync.dma_start(y2[:, :], out_sb)

    nc.finalize()
    return nc


def kernel(input, A, descendance):
    input = np.asarray(input)
    A = np.asarray(A, dtype=np.float32)
    desc = np.asarray(descendance).astype(np.int64)

    matrix_in = np.ascontiguousarray(input, dtype=np.float32).reshape(PARENT, C)
    up = matrix_in[desc]  # (CHILD, C) gather

    if "nc" not in _CACHE:
        _CACHE["nc"] = _build_program()
    nc = _CACHE["nc"]

    # Shard: core k gets at = A[:, k*JPC:(k+1)*JPC].T  (contiguous (JPC, CHILD))
    at_all = np.ascontiguousarray(
        A.reshape(CHILD, NCORES, JPC).transpose(1, 2, 0)
    )  # (NCORES, JPC, CHILD)
    in_maps = []
    for k in range(NCORES):
        in_maps.append(
            {
                "at": at_all[k],
                "u": np.ascontiguousarray(up[k * JPC : (k + 1) * JPC]),
            }
        )

    res = run_bass_kernel_spmd(nc, in_maps, core_ids=list(range(NCORES)))
    outs = res.results

    acc = np.zeros((128, NIB * C), dtype=np.float64)
    for k in range(NCORES):
        acc += outs[k]["y2"]
    # Unscramble: y2[p, ib*32+c] -> Y[ib*128+p, c]
    Y = (
        acc.reshape(128, NIB, C)
        .transpose(1, 0, 2)
        .reshape(CHILD, C)
        .astype(np.float32)
    )
    return Y.reshape(1, C, CHILD)


# revision 11
# speedup vs baseline: 40.6343x; 2.2312x over previous
"""GraphUpsampling kernel for 8x TRN2 NeuronCores.

Math: out = (A / colsum(A)) @ input.reshape(P,C)[descendance]
    == A @ (up / colsum(A)[:,None])          (scale the small side)

Sharding: COLUMN-shard A across 8 cores. Core k owns columns
j in [k*1024, (k+1)*1024). Each core holds the FULL column, so it
computes its own colsum locally -- zero communication. Each core
produces a partial output (8192, 32) = A[:, jk] @ up_scaled[jk]; the
host sums the 8 partials (the unshard reduction).

Device layout: core k's A slice is pre-transposed on host to
at = A[:, jk].T with shape (1024, 8192), so the contraction dim j is
the SBUF partition dim -- no on-chip transpose needed, colsum is a
free-dim vector reduce, and matmul uses at tiles as stationary lhsT.

PSUM: the full partial output (64 i-blocks x [128, 32]) is packed into
4 PSUM banks. A start=True matmul clears has_written bits bank-wide,
so we zero each bank once with a cheap K=1 all-zeros matmul and run
every real matmul with start=False (pure accumulate).
"""

import sys

sys.path.insert(0, "/opt/trn_rl_repo")

import numpy as np

import concourse.bass as bass
import concourse.mybir as mybir
from concourse import bacc
from concourse.bass_utils import run_bass_kernel_spmd
from concourse.tile import TileContext

PARENT = 4096
CHILD = 8192
C = 32
NCORES = 8
JPC = CHILD // NCORES  # 1024 columns of A per core
NSTRIPE = JPC // 128  # 8 stripes of 128 j per core
NIB = CHILD // 128  # 64 i-blocks of 128

_CACHE = {}


def _build_program(repeats=1):
    fp32 = mybir.dt.float32
    nc = bacc.Bacc("TRN2", target_bir_lowering=False)
    at = nc.dram_tensor("at", (JPC, CHILD), fp32, kind="ExternalInput")
    u = nc.dram_tensor("u", (JPC, C), fp32, kind="ExternalInput")
    # Output in scrambled layout [128, 64*32]: y2[p, ib*32+c] = Y[ib*128+p, c].
    # Host unscrambles; this keeps the store DMA contiguous (8KB/partition).
    y2 = nc.dram_tensor("y2", (128, NIB * C), fp32, kind="ExternalOutput")

    with TileContext(nc) as tc:
        with (
            tc.tile_pool(name="stripes", bufs=3) as spool,
            tc.tile_pool(name="small", bufs=1) as small,
            tc.tile_pool(name="uscaled", bufs=NSTRIPE) as upool,
            tc.tile_pool(name="stats", bufs=NSTRIPE) as stpool,
            tc.tile_pool(name="psum", bufs=1, space="PSUM") as ppool,
            tc.tile_pool(name="evict", bufs=1) as epool,
        ):
            zlhs = small.tile([1, 128], fp32, tag="zlhs")
            nc.vector.memset(zlhs, 0.0)
            zrhs = small.tile([1, 512], fp32, tag="zrhs")
            nc.vector.memset(zrhs, 0.0)

            for rep in range(repeats):
                psum_out = ppool.tile([128, NIB * C], fp32)  # 2048 fp32 = 4 banks
                # Zero all 4 banks + set every has_written bit (K=1 matmul).
                for b in range(4):
                    nc.tensor.matmul(
                        psum_out[:, b * 512 : (b + 1) * 512],
                        zlhs[:, :],
                        zrhs[:, :],
                        start=True,
                        stop=False,
                        skip_group_check=True,
                    )

                for jc in range(NSTRIPE):
                    stripe = spool.tile([128, CHILD], fp32, tag="stripe")
                    nc.sync.dma_start(stripe, at[jc * 128 : (jc + 1) * 128, :])
                    s = stpool.tile([128, 1], fp32, tag="s")
                    nc.vector.reduce_sum(s, stripe, axis=mybir.AxisListType.X)
                    r = stpool.tile([128, 1], fp32, tag="r")
                    nc.vector.reciprocal(r, s)
                    uc = upool.tile([128, C], fp32, tag="uc")
                    nc.sync.dma_start(uc, u[jc * 128 : (jc + 1) * 128, :])
                    us = upool.tile([128, C], fp32, tag="us")
                    nc.scalar.activation(
                        us, uc, mybir.ActivationFunctionType.Copy, scale=r
                    )
                    last = jc == NSTRIPE - 1
                    # outT[c, i] packed: i-chunk q (512 wide) -> bank b=q//4,
                    # col-group g=q%4 at psum partitions [32g, 32g+32).
                    # us is stationary (32 cols), at-stripe chunks are moving
                    # (N=512) -- avoids a 128-col LDWEIGHTS per matmul.
                    for q in range(CHILD // 512):
                        b, g = divmod(q, 4)
                        nc.tensor.matmul(
                            psum_out[32 * g : 32 * (g + 1), b * 512 : (b + 1) * 512],
                            us[:, :],
                            stripe[:, q * 512 : (q + 1) * 512],
                            start=False,
                            stop=last,
                            skip_group_check=True,
                            tile_position=(0, 32 * g),
                        )

                out_sb = epool.tile([128, NIB * C], fp32)
                nc.scalar.copy(out_sb, psum_out)
                nc.sync.dma_start(y2[:, :], out_sb)

    nc.finalize()
    return nc


def kernel(input, A, descendance):
    input = np.asarray(input)
    A = np.asarray(A, dtype=np.float32)
    desc = np.asarray(descendance).astype(np.int64)

    matrix_in = np.ascontiguousarray(input, dtype=np.float32).reshape(PARENT, C)
    up = matrix_in[desc]  # (CHILD, C) gather

    if "nc" not in _CACHE:
        _CACHE["nc"] = _build_program()
    nc = _CACHE["nc"]

    # Shard: core k gets at = A[:, k*JPC:(k+1)*JPC].T  (contiguous (JPC, CHILD))
    at_all = np.ascontiguousarray(
        A.reshape(CHILD, NCORES, JPC).transpose(1, 2, 0)
    )  # (NCORES, JPC, CHILD)
    in_maps = []
    for k in range(NCORES):
        in_maps.append(
            {
                "at": at_all[k],
                "u": np.ascontiguousarray(up[k * JPC : (k + 1) * JPC]),
            }
        )

    res = run_bass_kernel_spmd(nc, in_maps, core_ids=list(range(NCORES)))
    outs = res.results

    acc = np.zeros((128, NIB * C), dtype=np.float64)
    for k in range(NCORES):
        acc += outs[k]["y2"]
    # Unscramble: y2[32g+c, 512b+o] -> Y[(4b+g)*512+o, c]
    Y = (
        acc.reshape(4, C, 4, 512)
        .transpose(2, 0, 3, 1)
        .reshape(CHILD, C)
        .astype(np.float32)
    )
    return Y.reshape(1, C, CHILD)
